# revision 4
# baseline (speedup 1.0000x reference)
"""MACE MessagePassingConvolution on 8 Trainium2 NeuronCores.

Strategy (graph/data parallel over edges, sorted by receiver):
  - Host: sort edges by receiver, split into 8 contiguous chunks at node
    boundaries.  Within a core, group edges into variable-width node
    "windows" (<=128 nodes and <=T_TILES*128 edges each), padded to a
    static grid of W windows x T_TILES tiles of 128 edges.
  - Device, per 128-edge tile: indirect-DMA gather of sender features,
    radial-MLP gates (PE matmuls, edges on the free dim; the orientation
    flips at the last layer by making h3^T the stationary operand), message
    construction with fused vector ops (all CG / spherical-harmonic /
    normalization constants pre-folded into w3's columns on the host), and
    a one-hot selection matmul that scatter-accumulates the 1408-wide
    messages into PSUM window accumulators.
  - Each window drains PSUM -> SBUF -> DRAM rows at static offsets; the
    host reassembles per-core slabs into the full [N, 1408] output.
"""

import os
import sys

for _p in ("/opt/trn_rl_repo", "/root/.axon_site/_ro/trn_rl_repo"):
    if os.path.isdir(_p) and _p not in sys.path:
        sys.path.insert(0, _p)

import numpy as np

# ---------------------------------------------------------------- constants
C = 128
N_NODES = 10000
N_EDGES = 100000
R_DIM = 8
NCORES = 8
P = 128

T_TILES = 8                 # 128-edge tiles per window (must be mult of 4)
EDGE_CAP = T_TILES * P      # max edges per window
CHUNK = 512                 # edges per radial-MLP chunk (4 tiles)

AVG_NUM_NEIGHBORS = 10.0
SH_L2 = np.sqrt(7.5)
CG_121 = np.sqrt(0.4)
# CG_011*SH_L1 == 1 and CG_110*SH_L1 == 1 exactly, so tp0/tp1 need no
# extra scale; tp2's scale is CG_121*SH_L2 == sqrt(3).
G4_SCALE = CG_121 * SH_L2

_CACHE: dict = {}


def _install_ntff_hook():
    """Make run_bass_kernel_spmd(trace=True) usable under axon: the agent
    image's antenv lacks axon_hooks, so recreate the registry and install
    the ctypes NTFF hook. Harmless if profiling is never requested."""
    import types

    if "antenv.axon_hooks" in sys.modules:
        return
    mod = types.ModuleType("antenv.axon_hooks")
    _h = [None]
    mod.set_axon_ntff_profile_hook = lambda h: _h.__setitem__(0, h)
    mod.get_axon_ntff_profile_hook = lambda: _h[0]
    sys.modules["antenv.axon_hooks"] = mod
    try:
        import antenv

        antenv.axon_hooks = mod
    except ImportError:
        pass
    try:
        from trn_agent_boot.trn_boot import _ntff_profile_via_ctypes

        h = _ntff_profile_via_ctypes("/opt/axon/libaxon_pjrt.so")
        if h is not None:
            mod.set_axon_ntff_profile_hook(h)
    except Exception:
        pass


# ---------------------------------------------------------------- host prep
def _make_windows(senders, receivers, n_nodes):
    """Sort edges by receiver, shard into NCORES chunks at node boundaries,
    then split each core's node range into windows of <=128 nodes and
    <=EDGE_CAP edges."""
    E = senders.shape[0]
    perm = np.argsort(receivers, kind="stable")
    recv_s = receivers[perm]

    splits = [0]
    for c in range(1, NCORES):
        t = (c * E) // NCORES
        while 0 < t < E and recv_s[t] == recv_s[t - 1]:
            t += 1
        splits.append(t)
    splits.append(E)

    deg = np.bincount(receivers, minlength=n_nodes)

    cores = []
    for c in range(NCORES):
        e0, e1 = splits[c], splits[c + 1]
        node_lo = 0 if c == 0 else (int(recv_s[e0]) if e0 < E else n_nodes)
        node_hi = int(recv_s[e1]) if e1 < E else n_nodes
        if c == NCORES - 1:
            node_hi = n_nodes
        if e0 == e1:
            node_lo = node_hi
        wins = []  # (node_start, node_len, edge_start, edge_count)
        n, e = node_lo, e0
        while n < node_hi:
            ns, ecnt = n, 0
            while n < node_hi and n - ns < P:
                d = int(deg[n])
                if ecnt + d > EDGE_CAP:
                    break
                ecnt += d
                n += 1
            assert n > ns, f"node {ns} degree {deg[ns]} exceeds window cap"
            wins.append((ns, n - ns, e, ecnt))
            e += ecnt
        assert e == e1, (c, e, e1)
        cores.append({"wins": wins})
    return perm, cores


def _prep_core_arrays(W, wins, perm, senders, vectors, radial):
    """Build the padded per-core device arrays for a static W x T_TILES grid."""
    L = W * EDGE_CAP
    snd = np.zeros(L, np.int32)
    rrv = np.full(L, -1.0, np.float32)
    vec = np.zeros((L, 3), np.float32)
    vec[:, 0] = 1.0  # pad vectors normalize safely
    rad = np.zeros((L, R_DIM), np.float32)
    recv = None
    for w, (ns, nl, es, ec) in enumerate(wins):
        o = w * EDGE_CAP
        idx = perm[es:es + ec]
        snd[o:o + ec] = senders[idx]
        if recv is None:
            recv = True
        rrv[o:o + ec] = (  # relative receiver row within the window
            _RECV_SORTED[es:es + ec] - ns).astype(np.float32)
        vec[o:o + ec] = vectors[idx]
        rad[o:o + ec] = radial[idx]
    # per-window [128, T] layouts for single-DMA window loads
    sndT = snd.reshape(W, T_TILES, P).transpose(0, 2, 1).copy()
    rrvT = rrv.reshape(W, T_TILES, P).transpose(0, 2, 1).copy()
    # vec: [W, T, P, 3] -> [W, P, T, 3] so tile t is cols 3t:3t+3
    vecT = vec.reshape(W, T_TILES, P, 3).transpose(0, 2, 1, 3).reshape(
        W, P, 3 * T_TILES).copy()
    # radial: per 512-edge chunk transposed to [8, 512]
    radT = rad.reshape(L // CHUNK, CHUNK, R_DIM).transpose(0, 2, 1).copy()
    return sndT, rrvT, vecT, radT


_RECV_SORTED = None


# ---------------------------------------------------------------- bass build
def _build_module(W):
    import concourse.bass as bass
    import concourse.mybir as mybir
    import concourse.tile as tile
    from concourse import bacc
    from concourse.alu_op_type import AluOpType

    f32 = mybir.dt.float32
    i32 = mybir.dt.int32
    AF = mybir.ActivationFunctionType
    X = mybir.AxisListType.X

    nc = bacc.Bacc("TRN2")

    nf_d = nc.dram_tensor("node_feats", [N_NODES, 4 * C], f32,
                          kind="ExternalInput")
    w0_d = nc.dram_tensor("w0p", [R_DIM, 64], f32, kind="ExternalInput")
    w1_d = nc.dram_tensor("w1p", [64, 64], f32, kind="ExternalInput")
    w2_d = nc.dram_tensor("w2p", [64, 64], f32, kind="ExternalInput")
    w3a_d = nc.dram_tensor("w3a", [64, 512], f32, kind="ExternalInput")
    w3b_d = nc.dram_tensor("w3b", [64, 128], f32, kind="ExternalInput")
    iota_d = nc.dram_tensor("iota", [P, P], f32, kind="ExternalInput")
    snd_d = nc.dram_tensor("snd", [W, P, T_TILES], i32, kind="ExternalInput")
    rrv_d = nc.dram_tensor("rrv", [W, P, T_TILES], f32, kind="ExternalInput")
    vec_d = nc.dram_tensor("vec", [W, P, 3 * T_TILES], f32,
                           kind="ExternalInput")
    radT_d = nc.dram_tensor("radT", [W * EDGE_CAP // CHUNK, R_DIM, CHUNK],
                            f32, kind="ExternalInput")
    out_d = nc.dram_tensor("out", [W * P, 11 * C], f32, kind="ExternalOutput")

    with tile.TileContext(nc) as tc:
        with (
            tc.tile_pool(name="const", bufs=1) as cp,
            tc.tile_pool(name="win", bufs=2) as wp,
            tc.tile_pool(name="chunk", bufs=2) as kp,
            tc.tile_pool(name="tp", bufs=3) as tp,
            tc.tile_pool(name="pwin", bufs=1, space="PSUM") as pwin,
            tc.tile_pool(name="pmix", bufs=2, space="PSUM") as pmix,
            tc.tile_pool(name="ph", bufs=1, space="PSUM") as ph,
        ):
            w0sb = cp.tile([R_DIM, 64], f32)
            nc.sync.dma_start(out=w0sb[:], in_=w0_d[:])
            w1sb = cp.tile([64, 64], f32)
            nc.sync.dma_start(out=w1sb[:], in_=w1_d[:])
            w2sb = cp.tile([64, 64], f32)
            nc.sync.dma_start(out=w2sb[:], in_=w2_d[:])
            w3asb = cp.tile([64, 512], f32)
            nc.sync.dma_start(out=w3asb[:], in_=w3a_d[:])
            w3bsb = cp.tile([64, 128], f32)
            nc.sync.dma_start(out=w3bsb[:], in_=w3b_d[:])
            iotasb = cp.tile([P, P], f32)
            nc.sync.dma_start(out=iotasb[:], in_=iota_d[:])

            for w in range(W):
                sndw = wp.tile([P, T_TILES], i32, tag="sndw")
                nc.sync.dma_start(out=sndw[:], in_=snd_d[w])
                rrvw = wp.tile([P, T_TILES], f32, tag="rrvw")
                nc.sync.dma_start(out=rrvw[:], in_=rrv_d[w])
                vecw = wp.tile([P, 3 * T_TILES], f32, tag="vecw")
                nc.sync.dma_start(out=vecw[:], in_=vec_d[w])

                pw = pwin.tile([P, 11 * C], f32, tag="pw")

                for half in range(2):
                    ck = w * 2 + half
                    radt = kp.tile([R_DIM, CHUNK], f32, tag="radt")
                    nc.sync.dma_start(out=radt[:], in_=radT_d[ck])
                    h1p = ph.tile([64, CHUNK], f32, tag="hp")
                    nc.tensor.matmul(h1p[:], w0sb[:], radt[:],
                                     start=True, stop=True)
                    h1s = kp.tile([64, CHUNK], f32, tag="h1s")
                    nc.scalar.activation(h1s[:], h1p[:], AF.Silu)
                    h2p = ph.tile([64, CHUNK], f32, tag="hp")
                    nc.tensor.matmul(h2p[:], w1sb[:], h1s[:],
                                     start=True, stop=True)
                    h2s = kp.tile([64, CHUNK], f32, tag="h2s")
                    nc.scalar.activation(h2s[:], h2p[:], AF.Silu)
                    h3p = ph.tile([64, CHUNK], f32, tag="hp")
                    nc.tensor.matmul(h3p[:], w2sb[:], h2s[:],
                                     start=True, stop=True)
                    h3s = kp.tile([64, CHUNK], f32, tag="h3s")
                    nc.scalar.activation(h3s[:], h3p[:], AF.Silu)

                    for t4 in range(4):
                        tt = half * 4 + t4
                        h3sl = h3s[:, t4 * P:(t4 + 1) * P]
                        mix = pmix.tile([P, 640], f32, tag="mix")
                        nc.tensor.matmul(mix[:, 0:512], h3sl, w3asb[:],
                                         start=True, stop=True)
                        nc.tensor.matmul(mix[:, 512:640], h3sl, w3bsb[:],
                                         start=True, stop=True)
                        g0 = mix[:, 0:128]
                        g1 = mix[:, 128:256]
                        g2 = mix[:, 256:384]
                        g3 = mix[:, 384:512]
                        g4 = mix[:, 512:640]

                        nf = tp.tile([P, 4 * C], f32, tag="nf")
                        nc.gpsimd.indirect_dma_start(
                            out=nf[:], out_offset=None,
                            in_=nf_d[:],
                            in_offset=bass.IndirectOffsetOnAxis(
                                ap=sndw[:, tt:tt + 1], axis=0),
                        )
                        ss = nf[:, 0:C]
                        vs3 = nf[:, C:4 * C].rearrange(
                            "p (c i) -> p c i", i=3)

                        v3 = vecw[:, 3 * tt:3 * tt + 3]
                        sq = tp.tile([P, 3], f32, tag="sq")
                        nc.vector.tensor_tensor(sq[:], v3, v3,
                                                op=AluOpType.mult)
                        s1 = tp.tile([P, 1], f32, tag="s1")
                        nc.vector.reduce_sum(s1[:], sq[:], axis=X)
                        sr = tp.tile([P, 1], f32, tag="sr")
                        nc.scalar.activation(sr[:], s1[:], AF.Sqrt)
                        rinv = tp.tile([P, 1], f32, tag="rinv")
                        nc.vector.reciprocal(rinv[:], sr[:])
                        rn = tp.tile([P, 3], f32, tag="rn")
                        nc.vector.tensor_tensor(rn[:], v3,
                                                rinv[:].to_broadcast([P, 3]),
                                                op=AluOpType.mult)
                        rnb = rn[:].unsqueeze(1).to_broadcast([P, C, 3])

                        # t0[c] = sum_j vs[c,j]*rn[j]
                        mscr = tp.tile([P, 3 * C], f32, tag="mscr")
                        m3 = mscr[:].rearrange("p (c i) -> p c i", i=3)
                        nc.vector.tensor_tensor(m3, vs3, rnb,
                                                op=AluOpType.mult)
                        t0 = tp.tile([P, C], f32, tag="t0")
                        nc.vector.reduce_sum(t0[:], m3, axis=X)

                        msg = tp.tile([P, 11 * C], f32, tag="msg")
                        # scalars: [ss*g0 | t0*g1]
                        nc.vector.tensor_tensor(msg[:, 0:128], ss, g0,
                                                op=AluOpType.mult)
                        nc.vector.tensor_tensor(msg[:, 128:256], t0[:], g1,
                                                op=AluOpType.mult)
                        # vectors: vs*g2
                        nc.vector.tensor_tensor(
                            msg[:, 256:640].rearrange("p (c i) -> p c i", i=3),
                            vs3,
                            g2.unsqueeze(2).to_broadcast([P, C, 3]),
                            op=AluOpType.mult)
                        # tp1 = (ss*g3) x rn
                        aa = tp.tile([P, C], f32, tag="aa")
                        nc.vector.tensor_tensor(aa[:], ss, g3,
                                                op=AluOpType.mult)
                        nc.vector.tensor_tensor(
                            msg[:, 640:1024].rearrange("p (c i) -> p c i", i=3),
                            aa[:].unsqueeze(2).to_broadcast([P, C, 3]),
                            rnb, op=AluOpType.mult)
                        # tp2 = (t0*g4) x rn - (vs*g4)/3
                        dd = tp.tile([P, C], f32, tag="dd")
                        nc.vector.tensor_tensor(dd[:], t0[:], g4,
                                                op=AluOpType.mult)
                        ee = tp.tile([P, 3 * C], f32, tag="ee")
                        nc.vector.tensor_tensor(
                            ee[:].rearrange("p (c i) -> p c i", i=3),
                            dd[:].unsqueeze(2).to_broadcast([P, C, 3]),
                            rnb, op=AluOpType.mult)
                        bb = tp.tile([P, 3 * C], f32, tag="bb")
                        nc.vector.tensor_tensor(
                            bb[:].rearrange("p (c i) -> p c i", i=3),
                            vs3,
                            g4.unsqueeze(2).to_broadcast([P, C, 3]),
                            op=AluOpType.mult)
                        nc.vector.scalar_tensor_tensor(
                            out=msg[:, 1024:1408],
                            in0=bb[:], scalar=-1.0 / 3.0, in1=ee[:],
                            op0=AluOpType.mult, op1=AluOpType.add)

                        sel = tp.tile([P, P], f32, tag="sel")
                        nc.vector.tensor_tensor(
                            sel[:],
                            rrvw[:, tt:tt + 1].to_broadcast([P, P]),
                            iotasb[:], op=AluOpType.is_equal)

                        st = (tt == 0)
                        sp = (tt == T_TILES - 1)
                        nc.tensor.matmul(pw[:, 0:512], sel[:], msg[:, 0:512],
                                         start=st, stop=sp,
                                         skip_group_check=True)
                        nc.tensor.matmul(pw[:, 512:1024], sel[:],
                                         msg[:, 512:1024],
                                         start=st, stop=sp,
                                         skip_group_check=True)
                        nc.tensor.matmul(pw[:, 1024:1408], sel[:],
                                         msg[:, 1024:1408],
                                         start=st, stop=sp,
                                         skip_group_check=True)

                outsb = wp.tile([P, 11 * C], f32, tag="outsb")
                nc.scalar.copy(outsb[:, 0:512], pw[:, 0:512])
                nc.scalar.copy(outsb[:, 512:1024], pw[:, 512:1024])
                nc.scalar.copy(outsb[:, 1024:1408], pw[:, 1024:1408])
                nc.sync.dma_start(out=out_d[w * P:(w + 1) * P, :],
                                  in_=outsb[:])

    nc.finalize()
    return nc


# ---------------------------------------------------------------- entry
def _prepare_weights(w0, w1, w2, w3):
    w0p = (w0 / np.sqrt(8.0)).astype(np.float32)
    w1p = (w1 / 8.0).astype(np.float32)
    w2p = (w2 / 8.0).astype(np.float32)
    w3p = (w3 / 8.0 / np.sqrt(AVG_NUM_NEIGHBORS)).astype(np.float32).copy()
    w3p[:, 4 * C:5 * C] *= G4_SCALE
    return w0p, w1p, w2p, w3p[:, 0:512].copy(), w3p[:, 512:640].copy()


def kernel(vectors, node_feats, radial_embedding, w0, w1, w2, w3, senders,
           receivers):
    global _RECV_SORTED
    _install_ntff_hook()
    from concourse.bass_utils import run_bass_kernel_spmd

    vectors = np.asarray(vectors, np.float32)
    node_feats = np.asarray(node_feats, np.float32)
    radial = np.asarray(radial_embedding, np.float32)
    senders = np.asarray(senders, np.int32)
    receivers = np.asarray(receivers, np.int32)

    perm, cores = _make_windows(senders, receivers, N_NODES)
    _RECV_SORTED = receivers[perm].astype(np.int64)
    W = max(len(c["wins"]) for c in cores)

    key = ("mod", W)
    if key not in _CACHE:
        _CACHE[key] = _build_module(W)
    nc = _CACHE[key]

    w0p, w1p, w2p, w3a, w3b = _prepare_weights(w0, w1, w2, w3)
    iota = np.broadcast_to(np.arange(P, dtype=np.float32), (P, P)).copy()

    in_maps = []
    for c in range(NCORES):
        sndT, rrvT, vecT, radT = _prep_core_arrays(
            W, cores[c]["wins"], perm, senders, vectors, radial)
        in_maps.append({
            "node_feats": node_feats, "w0p": w0p, "w1p": w1p, "w2p": w2p,
            "w3a": w3a, "w3b": w3b, "iota": iota,
            "snd": sndT, "rrv": rrvT, "vec": vecT, "radT": radT,
        })

    global _LAST_IN_MAPS
    _LAST_IN_MAPS = in_maps
    res = run_bass_kernel_spmd(nc, in_maps, core_ids=list(range(NCORES)))

    out = np.zeros((N_NODES, 11 * C), np.float32)
    for c in range(NCORES):
        co = res.results[c]["out"]
        for w, (ns, nl, _es, _ec) in enumerate(cores[c]["wins"]):
            out[ns:ns + nl] = co[w * P:w * P + nl]
    return out


# revision 7
# speedup vs baseline: 1.5966x; 1.5966x over previous
"""MACE MessagePassingConvolution on 8 Trainium2 NeuronCores.

Strategy (graph/data parallel over edges, sorted by receiver):
  - Host: sort edges by receiver, split into 8 contiguous chunks at node
    boundaries.  Within a core, group edges into variable-width node
    "windows" (<=128 nodes and <=T_TILES*128 edges each), padded to a
    static grid of W windows x T_TILES tiles of 128 edges.
  - Device, per 128-edge tile: dma_gather of sender features (bf16),
    radial-MLP gates (PE matmuls, edges on the free dim; the orientation
    flips at the last layer by making h3^T the stationary operand), message
    construction with chunk-batched vector ops (all CG / spherical-harmonic
    / normalization constants pre-folded into w3's columns on the host),
    and a one-hot selection matmul that scatter-accumulates the 1408-wide
    bf16 messages into fp32 PSUM window accumulators.
  - Each window drains PSUM -> SBUF -> DRAM rows at static offsets; the
    host reassembles per-core slabs into the full [N, 1408] output.
"""

import os
import sys

for _p in ("/opt/trn_rl_repo", "/root/.axon_site/_ro/trn_rl_repo"):
    if os.path.isdir(_p) and _p not in sys.path:
        sys.path.insert(0, _p)

import numpy as np

# ---------------------------------------------------------------- constants
C = 128
N_NODES = 10000
N_EDGES = 100000
R_DIM = 8
NCORES = 8
P = 128

T_TILES = 8                 # 128-edge tiles per window (must be mult of 4)
EDGE_CAP = T_TILES * P      # max edges per window
CHUNK = 512                 # edges per radial-MLP chunk (4 tiles)

AVG_NUM_NEIGHBORS = 10.0
SH_L2 = np.sqrt(7.5)
CG_121 = np.sqrt(0.4)
# CG_011*SH_L1 == 1 and CG_110*SH_L1 == 1 exactly, so tp0/tp1 need no
# extra scale; tp2's scale is CG_121*SH_L2 == sqrt(3).
G4_SCALE = CG_121 * SH_L2

_CACHE: dict = {}
_LAST_IN_MAPS = None


def _install_ntff_hook():
    """Make run_bass_kernel_spmd(trace=True) usable under axon: the agent
    image's antenv lacks axon_hooks, so recreate the registry and install
    the ctypes NTFF hook. Harmless if profiling is never requested."""
    import types

    if "antenv.axon_hooks" in sys.modules:
        return
    mod = types.ModuleType("antenv.axon_hooks")
    _h = [None]
    mod.set_axon_ntff_profile_hook = lambda h: _h.__setitem__(0, h)
    mod.get_axon_ntff_profile_hook = lambda: _h[0]
    sys.modules["antenv.axon_hooks"] = mod
    try:
        import antenv

        antenv.axon_hooks = mod
    except ImportError:
        pass
    try:
        from trn_agent_boot.trn_boot import _ntff_profile_via_ctypes

        h = _ntff_profile_via_ctypes("/opt/axon/libaxon_pjrt.so")
        if h is not None:
            mod.set_axon_ntff_profile_hook(h)
    except Exception:
        pass


# ---------------------------------------------------------------- host prep
def _make_windows(senders, receivers, n_nodes):
    """Sort edges by receiver, shard into NCORES chunks at node boundaries,
    then split each core's node range into windows of <=128 nodes and
    <=EDGE_CAP edges."""
    E = senders.shape[0]
    perm = np.argsort(receivers, kind="stable")
    recv_s = receivers[perm]

    splits = [0]
    for c in range(1, NCORES):
        t = (c * E) // NCORES
        while 0 < t < E and recv_s[t] == recv_s[t - 1]:
            t += 1
        splits.append(t)
    splits.append(E)

    deg = np.bincount(receivers, minlength=n_nodes)

    cores = []
    for c in range(NCORES):
        e0, e1 = splits[c], splits[c + 1]
        node_lo = 0 if c == 0 else (int(recv_s[e0]) if e0 < E else n_nodes)
        node_hi = int(recv_s[e1]) if e1 < E else n_nodes
        if c == NCORES - 1:
            node_hi = n_nodes
        if e0 == e1:
            node_lo = node_hi
        wins = []  # (node_start, node_len, edge_start, edge_count)
        n, e = node_lo, e0
        while n < node_hi:
            ns, ecnt = n, 0
            while n < node_hi and n - ns < P:
                d = int(deg[n])
                if ecnt + d > EDGE_CAP:
                    break
                ecnt += d
                n += 1
            assert n > ns, f"node {ns} degree {deg[ns]} exceeds window cap"
            wins.append((ns, n - ns, e, ecnt))
            e += ecnt
        assert e == e1, (c, e, e1)
        cores.append({"wins": wins})
    return perm, recv_s, cores


def _prep_core_arrays(W, wins, perm, recv_s, senders, vectors, radial, bf16):
    """Padded per-core device arrays for a static W x T_TILES grid."""
    L = W * EDGE_CAP
    snd = np.zeros(L, np.int16)
    rrv = np.full(L, -1.0, np.float32)
    vec = np.zeros((L, 3), np.float32)
    vec[:, 0] = 1.0  # pad vectors normalize safely
    rad = np.zeros((L, R_DIM), np.float32)
    for w, (ns, nl, es, ec) in enumerate(wins):
        o = w * EDGE_CAP
        idx = perm[es:es + ec]
        snd[o:o + ec] = senders[idx].astype(np.int16)
        rrv[o:o + ec] = (recv_s[es:es + ec] - ns).astype(np.float32)
        vec[o:o + ec] = vectors[idx]
        rad[o:o + ec] = radial[idx]
    # senders: dma_gather wrapped layout [W, 128, EDGE_CAP//16]
    # idx k of a window lives at [k%16, k//16], replicated to 128 partitions
    sndW = snd.reshape(W, EDGE_CAP // 16, 16).transpose(0, 2, 1)  # [W,16,cap/16]
    sndT = np.tile(sndW, (1, 8, 1)).copy()                        # [W,128,...]
    rrvT = rrv.reshape(W, T_TILES, P).transpose(0, 2, 1).astype(bf16).copy()
    # vec: all windows in one [128, W*T*3] tensor (tile t of window w at
    # columns (w*T+t)*3 : +3)
    vecT = vec.reshape(W * T_TILES, P, 3).transpose(1, 0, 2).reshape(
        P, W * T_TILES * 3).copy()
    # radial: per 512-edge chunk transposed to [8, 512]
    radT = rad.reshape(L // CHUNK, CHUNK, R_DIM).transpose(0, 2, 1).astype(
        bf16).copy()
    return sndT, rrvT, vecT, radT


# ---------------------------------------------------------------- bass build
def _build_module(W):
    import concourse.bass as bass
    import concourse.mybir as mybir
    import concourse.tile as tile
    from concourse import bacc
    from concourse.alu_op_type import AluOpType

    f32 = mybir.dt.float32
    bf = mybir.dt.bfloat16
    i16 = mybir.dt.int16
    AF = mybir.ActivationFunctionType
    X = mybir.AxisListType.X
    NT = W * T_TILES  # total tiles

    nc = bacc.Bacc("TRN2")

    nf_d = nc.dram_tensor("node_feats", [N_NODES, 4 * C], bf,
                          kind="ExternalInput")
    w0_d = nc.dram_tensor("w0p", [R_DIM, 64], bf, kind="ExternalInput")
    w1_d = nc.dram_tensor("w1p", [64, 64], bf, kind="ExternalInput")
    w2_d = nc.dram_tensor("w2p", [64, 64], bf, kind="ExternalInput")
    w3a_d = nc.dram_tensor("w3a", [64, 512], bf, kind="ExternalInput")
    w3b_d = nc.dram_tensor("w3b", [64, 128], bf, kind="ExternalInput")
    iota_d = nc.dram_tensor("iota", [P, P], bf, kind="ExternalInput")
    snd_d = nc.dram_tensor("snd", [W, P, EDGE_CAP // 16], i16,
                           kind="ExternalInput")
    rrv_d = nc.dram_tensor("rrv", [W, P, T_TILES], bf, kind="ExternalInput")
    vec_d = nc.dram_tensor("vec", [P, NT * 3], f32, kind="ExternalInput")
    radT_d = nc.dram_tensor("radT", [W * EDGE_CAP // CHUNK, R_DIM, CHUNK],
                            bf, kind="ExternalInput")
    out_d = nc.dram_tensor("out", [W * P, 11 * C], f32, kind="ExternalOutput")

    with tile.TileContext(nc) as tc:
        with (
            tc.tile_pool(name="const", bufs=1) as cp,
            tc.tile_pool(name="win", bufs=2) as wp,
            tc.tile_pool(name="chunk", bufs=2) as kp,
            tc.tile_pool(name="tp", bufs=2) as tp,
            tc.tile_pool(name="pwin", bufs=1, space="PSUM") as pwin,
            tc.tile_pool(name="pmix", bufs=2, space="PSUM") as pmix,
            tc.tile_pool(name="ph", bufs=1, space="PSUM") as ph,
        ):
            w0sb = cp.tile([R_DIM, 64], bf)
            nc.sync.dma_start(out=w0sb[:], in_=w0_d[:])
            w1sb = cp.tile([64, 64], bf)
            nc.sync.dma_start(out=w1sb[:], in_=w1_d[:])
            w2sb = cp.tile([64, 64], bf)
            nc.sync.dma_start(out=w2sb[:], in_=w2_d[:])
            w3asb = cp.tile([64, 512], bf)
            nc.sync.dma_start(out=w3asb[:], in_=w3a_d[:])
            w3bsb = cp.tile([64, 128], bf)
            nc.sync.dma_start(out=w3bsb[:], in_=w3b_d[:])
            iotasb = cp.tile([P, P], bf)
            nc.sync.dma_start(out=iotasb[:], in_=iota_d[:])

            # --- batched edge-vector normalization (one Sqrt table load) ---
            vecall = cp.tile([P, NT * 3], f32)
            nc.sync.dma_start(out=vecall[:], in_=vec_d[:])
            sqall = cp.tile([P, NT * 3], f32)
            nc.vector.tensor_tensor(sqall[:], vecall[:], vecall[:],
                                    op=AluOpType.mult)
            s1all = cp.tile([P, NT], f32)
            nc.vector.reduce_sum(
                s1all[:], sqall[:].rearrange("p (t i) -> p t i", i=3), axis=X)
            srall = cp.tile([P, NT], f32)
            nc.scalar.activation(srall[:], s1all[:], AF.Sqrt)
            rinvall = cp.tile([P, NT], f32)
            nc.vector.reciprocal(rinvall[:], srall[:])

            for w in range(W):
                sndw = wp.tile([P, EDGE_CAP // 16], i16, tag="sndw")
                nc.sync.dma_start(out=sndw[:], in_=snd_d[w])
                rrvw = wp.tile([P, T_TILES], bf, tag="rrvw")
                nc.sync.dma_start(out=rrvw[:], in_=rrv_d[w])

                # gather all 8 tiles' sender features for this window
                nf8 = wp.tile([P, T_TILES, 4 * C], bf, tag="nf8")
                nc.gpsimd.dma_gather(
                    out_ap=nf8[:], in_ap=nf_d[:], idxs_ap=sndw[:],
                    num_idxs=EDGE_CAP, num_idxs_reg=EDGE_CAP,
                    elem_size=4 * C)

                pw = pwin.tile([P, 11 * C], f32, tag="pw")

                for half in range(2):
                    ck = w * 2 + half
                    hs = half * 4
                    radt = kp.tile([R_DIM, CHUNK], bf, tag="radt")
                    nc.sync.dma_start(out=radt[:], in_=radT_d[ck])
                    h1p = ph.tile([64, CHUNK], f32, tag="hp")
                    nc.tensor.matmul(h1p[:], w0sb[:], radt[:],
                                     start=True, stop=True)
                    h1s = kp.tile([64, CHUNK], bf, tag="h1s")
                    nc.scalar.activation(h1s[:], h1p[:], AF.Silu)
                    h2p = ph.tile([64, CHUNK], f32, tag="hp")
                    nc.tensor.matmul(h2p[:], w1sb[:], h1s[:],
                                     start=True, stop=True)
                    h2s = kp.tile([64, CHUNK], bf, tag="h2s")
                    nc.scalar.activation(h2s[:], h2p[:], AF.Silu)
                    h3p = ph.tile([64, CHUNK], f32, tag="hp")
                    nc.tensor.matmul(h3p[:], w2sb[:], h2s[:],
                                     start=True, stop=True)
                    h3s = kp.tile([64, CHUNK], bf, tag="h3s")
                    nc.scalar.activation(h3s[:], h3p[:], AF.Silu)

                    # gates for the chunk's 4 tiles -> bf16 SBUF
                    mixs = kp.tile([P, 4, 640], bf, tag="mixs")
                    for t4 in range(4):
                        h3sl = h3s[:, t4 * P:(t4 + 1) * P]
                        mix = pmix.tile([P, 640], f32, tag="mix")
                        nc.tensor.matmul(mix[:, 0:512], h3sl, w3asb[:],
                                         start=True, stop=True)
                        nc.tensor.matmul(mix[:, 512:640], h3sl, w3bsb[:],
                                         start=True, stop=True)
                        nc.scalar.copy(mixs[:, t4, :], mix[:])

                    # per-tile normalized edge vectors (bf16), per-partition
                    # scale via ACT
                    rn4 = tp.tile([P, 4, 3], bf, tag="rn4")
                    for t4 in range(4):
                        slot = ck * 4 + t4
                        nc.scalar.activation(
                            rn4[:, t4, :], vecall[:, slot * 3:slot * 3 + 3],
                            AF.Copy, scale=rinvall[:, slot:slot + 1])

                    vs4 = nf8[:, hs:hs + 4, C:4 * C].rearrange(
                        "p t (c i) -> p t c i", i=3)
                    ss4 = nf8[:, hs:hs + 4, 0:C]
                    rn4b = rn4[:].unsqueeze(2).to_broadcast([P, 4, C, 3])
                    g0 = mixs[:, :, 0:128]
                    g1 = mixs[:, :, 128:256]
                    g2 = mixs[:, :, 256:384]
                    g3 = mixs[:, :, 384:512]
                    g4 = mixs[:, :, 512:640]

                    # t0[c] = sum_j vs[c,j]*rn[j]
                    mscr = tp.tile([P, 4, C, 3], bf, tag="mscr")
                    nc.vector.tensor_tensor(mscr[:], vs4, rn4b,
                                            op=AluOpType.mult)
                    t04 = tp.tile([P, 4, C], bf, tag="t04")
                    with nc.allow_low_precision(
                            reason="3-element dot, bf16 ok"):
                        nc.vector.reduce_sum(t04[:], mscr[:], axis=X)

                    msg = tp.tile([P, 4, 11 * C], bf, tag="msg")
                    # scalar part: [ss*g0 | t0*g1]
                    nc.vector.tensor_tensor(msg[:, :, 0:128], ss4, g0,
                                            op=AluOpType.mult)
                    nc.vector.tensor_tensor(msg[:, :, 128:256], t04[:], g1,
                                            op=AluOpType.mult)
                    # vector part 1: vs*g2
                    nc.vector.tensor_tensor(
                        msg[:, :, 256:640].rearrange("p t (c i) -> p t c i",
                                                     i=3),
                        vs4, g2.unsqueeze(3).to_broadcast([P, 4, C, 3]),
                        op=AluOpType.mult)
                    # tp1 = (ss*g3) x rn ; tp2 = (t0*g4) x rn - (vs*g4)/3
                    ad4 = tp.tile([P, 4, 2 * C], bf, tag="ad4")
                    nc.vector.tensor_tensor(ad4[:, :, 0:128], ss4, g3,
                                            op=AluOpType.mult)
                    nc.vector.tensor_tensor(ad4[:, :, 128:256], t04[:], g4,
                                            op=AluOpType.mult)
                    nc.vector.tensor_tensor(
                        msg[:, :, 640:1408].rearrange("p t (c i) -> p t c i",
                                                      i=3),
                        ad4[:].unsqueeze(3).to_broadcast([P, 4, 2 * C, 3]),
                        rn4[:].unsqueeze(2).to_broadcast([P, 4, 2 * C, 3]),
                        op=AluOpType.mult)
                    bb4 = tp.tile([P, 4, 3 * C], bf, tag="bb4")
                    nc.vector.tensor_tensor(
                        bb4[:].rearrange("p t (c i) -> p t c i", i=3),
                        vs4, g4.unsqueeze(3).to_broadcast([P, 4, C, 3]),
                        op=AluOpType.mult)
                    nc.vector.scalar_tensor_tensor(
                        out=msg[:, :, 1024:1408],
                        in0=bb4[:], scalar=-1.0 / 3.0,
                        in1=msg[:, :, 1024:1408],
                        op0=AluOpType.mult, op1=AluOpType.add)

                    sel4 = tp.tile([P, 4, P], bf, tag="sel4")
                    nc.vector.tensor_tensor(
                        sel4[:],
                        rrvw[:, hs:hs + 4].unsqueeze(2).to_broadcast(
                            [P, 4, P]),
                        iotasb[:].unsqueeze(1).to_broadcast([P, 4, P]),
                        op=AluOpType.is_equal)

                    for t4 in range(4):
                        tt = hs + t4
                        st = (tt == 0)
                        sp = (tt == T_TILES - 1)
                        sl = sel4[:, t4, :]
                        nc.tensor.matmul(pw[:, 0:512], sl,
                                         msg[:, t4, 0:512],
                                         start=st, stop=sp,
                                         skip_group_check=True)
                        nc.tensor.matmul(pw[:, 512:1024], sl,
                                         msg[:, t4, 512:1024],
                                         start=st, stop=sp,
                                         skip_group_check=True)
                        nc.tensor.matmul(pw[:, 1024:1408], sl,
                                         msg[:, t4, 1024:1408],
                                         start=st, stop=sp,
                                         skip_group_check=True)

                outsb = wp.tile([P, 11 * C], f32, tag="outsb")
                nc.scalar.copy(outsb[:, 0:512], pw[:, 0:512])
                nc.scalar.copy(outsb[:, 512:1024], pw[:, 512:1024])
                nc.scalar.copy(outsb[:, 1024:1408], pw[:, 1024:1408])
                nc.sync.dma_start(out=out_d[w * P:(w + 1) * P, :],
                                  in_=outsb[:])

    nc.finalize()
    return nc


# ---------------------------------------------------------------- entry
def _prepare_weights(w0, w1, w2, w3, bf16):
    w0p = (w0 / np.sqrt(8.0)).astype(bf16)
    w1p = (w1 / 8.0).astype(bf16)
    w2p = (w2 / 8.0).astype(bf16)
    w3p = (w3 / 8.0 / np.sqrt(AVG_NUM_NEIGHBORS)).astype(np.float32).copy()
    w3p[:, 4 * C:5 * C] *= G4_SCALE
    return (w0p, w1p, w2p, w3p[:, 0:512].astype(bf16).copy(),
            w3p[:, 512:640].astype(bf16).copy())


def kernel(vectors, node_feats, radial_embedding, w0, w1, w2, w3, senders,
           receivers):
    global _LAST_IN_MAPS
    _install_ntff_hook()
    import ml_dtypes
    from concourse.bass_utils import run_bass_kernel_spmd

    bf16 = ml_dtypes.bfloat16
    vectors = np.asarray(vectors, np.float32)
    node_feats = np.asarray(node_feats, np.float32)
    radial = np.asarray(radial_embedding, np.float32)
    senders = np.asarray(senders, np.int32)
    receivers = np.asarray(receivers, np.int32)

    perm, recv_s, cores = _make_windows(senders, receivers, N_NODES)
    W = max(len(c["wins"]) for c in cores)

    key = ("mod", W)
    if key not in _CACHE:
        _CACHE[key] = _build_module(W)
    nc = _CACHE[key]

    w0p, w1p, w2p, w3a, w3b = _prepare_weights(w0, w1, w2, w3, bf16)
    iota = np.broadcast_to(np.arange(P, dtype=np.float32),
                           (P, P)).astype(bf16).copy()
    nf_bf = node_feats.astype(bf16)

    in_maps = []
    for c in range(NCORES):
        sndT, rrvT, vecT, radT = _prep_core_arrays(
            W, cores[c]["wins"], perm, recv_s, senders, vectors, radial, bf16)
        in_maps.append({
            "node_feats": nf_bf, "w0p": w0p, "w1p": w1p, "w2p": w2p,
            "w3a": w3a, "w3b": w3b, "iota": iota,
            "snd": sndT, "rrv": rrvT, "vec": vecT, "radT": radT,
        })

    _LAST_IN_MAPS = in_maps
    res = run_bass_kernel_spmd(nc, in_maps, core_ids=list(range(NCORES)))

    out = np.zeros((N_NODES, 11 * C), np.float32)
    for c in range(NCORES):
        co = res.results[c]["out"]
        for w, (ns, nl, _es, _ec) in enumerate(cores[c]["wins"]):
            out[ns:ns + nl] = co[w * P:w * P + nl]
    return out
